# revision 10
# baseline (speedup 1.0000x reference)
"""CFN cell on 8 TRN2 NeuronCores — tensor-parallel over H, bf16 matmuls.

v4: weight-stationary orientation. Each core owns H_LOC=256 hidden
columns (2 tiles of 128). Stationary operand = weight chunk
[K=128, M=128 h-cols]; moving operand = transposed activations
[K=128, N=512 batch]. PSUM output is [h, batch], so

  * the theta/eta biases are per-partition -> folded into ACTIVATE,
  * tanh(state) comes from slices of the already-loaded st pack,
  * 1280 N=512 matmuls/core (vs 1536 mixed) — all at PE line rate.

All matmul operands are bf16 (host-converted): halves HBM traffic to
~39 MB/core and enables FWL weight loads; PSUM accumulation stays fp32
so the output error (~5e-4) is far inside the 2e-2 gate.
"""

import numpy as np
import ml_dtypes
from contextlib import ExitStack

import concourse.bass as bass
import concourse.mybir as mybir
import concourse.tile as tile
from concourse import bacc
from concourse.bass_utils import run_bass_kernel_spmd

F32 = mybir.dt.float32
BF16 = mybir.dt.bfloat16
AF = mybir.ActivationFunctionType

B, D_IN, H, NCORES = 4096, 2048, 2048, 8
H_LOC = H // NCORES          # 256 -> 2 output tiles of 128
BG = 512                     # batch-group width (PSUM bank limit)
N_BG = B // BG               # 8
KT = D_IN // 128             # 16 contraction chunks per operand side

TRACE = False
LAST_RESULTS = None
_NC_CACHE = {}


def build(nc):
    st = nc.dram_tensor("st", [N_BG, 128, KT, BG], BF16, kind="ExternalInput").ap()
    xt = nc.dram_tensor("xt", [N_BG, 128, KT, BG], BF16, kind="ExternalInput").ap()
    sts = nc.dram_tensor("sts", [N_BG, 128, 2, BG], BF16, kind="ExternalInput").ap()
    wsu = nc.dram_tensor("wsu", [128, KT, 512], BF16, kind="ExternalInput").ap()
    wsx = nc.dram_tensor("wsx", [128, KT, 512], BF16, kind="ExternalInput").ap()
    wx = nc.dram_tensor("wx", [128, KT, 256], BF16, kind="ExternalInput").ap()
    bias = nc.dram_tensor("bias", [128, 4], F32, kind="ExternalInput").ap()
    out = nc.dram_tensor("h_out", [N_BG, 128, 2, BG], F32, kind="ExternalOutput").ap()

    with tile.TileContext(nc) as tc, ExitStack() as ctx:
        consts = ctx.enter_context(tc.tile_pool(name="consts", bufs=1))
        acts = ctx.enter_context(tc.tile_pool(name="acts", bufs=3))
        temps = ctx.enter_context(tc.tile_pool(name="temps", bufs=2))
        psum = ctx.enter_context(tc.tile_pool(name="psum", bufs=1, space="PSUM"))

        wsu_sb = consts.tile([128, KT, 512], BF16, tag="wsu")
        wsx_sb = consts.tile([128, KT, 512], BF16, tag="wsx")
        wx_sb = consts.tile([128, KT, 256], BF16, tag="wx")
        bias_sb = consts.tile([128, 4], F32, tag="bias")

        st_map, xt_map, sts_map = {}, {}, {}

        # DMA issue rings.  The 16 DMA engines pull from every active ring,
        # so aggregate HBM bandwidth scales with the number of rings kept
        # busy; round-robin in exact consumption order keeps delivery
        # aligned with what the PE needs next.  gpsimd is reserved for the
        # output writes in steady state (an out DMA waits on the epilogue
        # and would head-of-line-block window loads queued behind it).
        import itertools
        _rr = itertools.count()
        rings4 = [nc.sync, nc.gpsimd, nc.scalar]
        rings3 = [nc.sync, nc.scalar]

        def q4():
            return rings4[next(_rr) % 3]

        def load_window(g):
            xtw = acts.tile([128, KT, BG], BF16, tag="xtw", name=f"xtw{g}")
            for i, (c0, c1) in enumerate(((0, 8), (8, 16))):
                rings3[i].dma_start(out=xtw[:, c0:c1, :],
                                    in_=xt[g, :, c0:c1, :])
            xt_map[g] = xtw
            stw = acts.tile([128, KT, BG], BF16, tag="stw", name=f"stw{g}")
            for i, (c0, c1) in enumerate(((0, 8), (8, 16))):
                rings3[1 - i].dma_start(out=stw[:, c0:c1, :],
                                        in_=st[g, :, c0:c1, :])
            st_map[g] = stw
            stsw = acts.tile([128, 2, BG], BF16, tag="stsw", name=f"sts{g}")
            rings3[g % 2].dma_start(out=stsw, in_=sts[g])
            sts_map[g] = stsw

        # ── Startup choreography ────────────────────────────────────────
        # Consumption order: [xt0|wsx] pairs (input phase), wx, [st0|wsu]
        # pairs (state phase), then window 1.  Fine chunks at the front so
        # the first matmul can start ~1 chunk after DMA begins.
        fine = ((0, 1), (1, 4), (4, 8), (8, 12), (12, 16))
        xtw0 = acts.tile([128, KT, BG], BF16, tag="xtw", name="xtw0")
        xt_map[0] = xtw0
        for c0, c1 in fine:
            q4().dma_start(out=xtw0[:, c0:c1, :], in_=xt[0, :, c0:c1, :])
            q4().dma_start(out=wsx_sb[:, c0:c1, :], in_=wsx[:, c0:c1, :])
        q4().dma_start(out=wx_sb[:, 0:8, :], in_=wx[:, 0:8, :])
        q4().dma_start(out=wx_sb[:, 8:16, :], in_=wx[:, 8:16, :])
        stw0 = acts.tile([128, KT, BG], BF16, tag="stw", name="stw0")
        st_map[0] = stw0
        coarse = ((0, 4), (4, 8), (8, 12), (12, 16))
        for c0, c1 in coarse:
            q4().dma_start(out=stw0[:, c0:c1, :], in_=st[0, :, c0:c1, :])
            q4().dma_start(out=wsu_sb[:, c0:c1, :], in_=wsu[:, c0:c1, :])
        q4().dma_start(out=bias_sb, in_=bias)
        stsw0 = acts.tile([128, 2, BG], BF16, tag="stsw", name="sts0")
        q4().dma_start(out=stsw0, in_=sts[0])
        sts_map[0] = stsw0
        xtw1 = acts.tile([128, KT, BG], BF16, tag="xtw", name="xtw1")
        for c0, c1 in ((0, 8), (8, 16)):
            q4().dma_start(out=xtw1[:, c0:c1, :], in_=xt[1, :, c0:c1, :])
        xt_map[1] = xtw1
        stw1 = acts.tile([128, KT, BG], BF16, tag="stw", name="stw1")
        for c0, c1 in ((0, 8), (8, 16)):
            q4().dma_start(out=stw1[:, c0:c1, :], in_=st[1, :, c0:c1, :])
        st_map[1] = stw1
        stsw1 = acts.tile([128, 2, BG], BF16, tag="stsw", name="sts1")
        q4().dma_start(out=stsw1, in_=sts[1])
        sts_map[1] = stsw1

        def group(g):
            # Both h-tiles share every window / weight chunk, so running
            # them in one pass halves the DMA demand per PE-second — the
            # only thing that matters while the weights still stream in
            # (group 0 would otherwise need ~520 GB/s vs ~320 available).
            # ht-major sub-loops release each PSUM bank ~10 µs before the
            # group ends, so bufs=1 per tag never stalls a boundary.
            stw, xtw, stsw = st_map[g], xt_map[g], sts_map[g]
            hs = [slice(0, 128), slice(128, 256)]
            es = [slice(256, 384), slice(384, 512)]
            th_ps = [psum.tile([128, BG], F32, tag=f"th{ht}", bufs=1,
                               name=f"th{g}_{ht}") for ht in range(2)]
            et_ps = [psum.tile([128, BG], F32, tag=f"et{ht}", bufs=1,
                               name=f"et{g}_{ht}") for ht in range(2)]
            wx_ps = [psum.tile([128, BG], F32, tag=f"wx{ht}", bufs=1,
                               name=f"wx{g}_{ht}") for ht in range(2)]
            for ht in range(2):
                for k in range(KT):
                    nc.tensor.matmul(th_ps[ht], wsx_sb[:, k, hs[ht]],
                                     xtw[:, k, :], start=(k == 0), stop=False)
            for ht in range(2):
                for k in range(KT):
                    nc.tensor.matmul(et_ps[ht], wsx_sb[:, k, es[ht]],
                                     xtw[:, k, :], start=(k == 0), stop=False)
            for ht in range(2):
                for k in range(KT):
                    nc.tensor.matmul(wx_ps[ht], wx_sb[:, k, hs[ht]],
                                     xtw[:, k, :], start=(k == 0),
                                     stop=(k == KT - 1))
            for ht in range(2):
                for k in range(KT):
                    nc.tensor.matmul(th_ps[ht], wsu_sb[:, k, hs[ht]],
                                     stw[:, k, :], start=False,
                                     stop=(k == KT - 1))
            for ht in range(2):
                for k in range(KT):
                    nc.tensor.matmul(et_ps[ht], wsu_sb[:, k, es[ht]],
                                     stw[:, k, :], start=False,
                                     stop=(k == KT - 1))

            # epilogue, scalar-queue ops emitted in availability order so a
            # late PSUM (et) never head-of-line-blocks an earlier one
            ths, twx, th, p1 = [], [], [], []
            for ht in range(2):
                ths.append(temps.tile([128, BG], F32, tag="ths",
                                      name=f"ths{g}_{ht}"))
                nc.scalar.activation(ths[ht], stsw[:, ht, :], AF.Tanh)
            for ht in range(2):
                twx.append(temps.tile([128, BG], F32, tag="twx",
                                      name=f"twx{g}_{ht}"))
                nc.scalar.activation(twx[ht], wx_ps[ht], AF.Tanh)
            for ht in range(2):
                th.append(temps.tile([128, BG], F32, tag="th_s",
                                     name=f"ths_{g}_{ht}"))
                nc.scalar.activation(th[ht], th_ps[ht], AF.Sigmoid,
                                     bias=bias_sb[:, ht:ht + 1])
                p1.append(temps.tile([128, BG], F32, tag="p1",
                                     name=f"p1{g}_{ht}"))
                nc.vector.tensor_mul(p1[ht], th[ht], ths[ht])
            for ht in range(2):
                et = temps.tile([128, BG], F32, tag="et_s", name=f"ets_{g}_{ht}")
                nc.scalar.activation(et, et_ps[ht], AF.Sigmoid,
                                     bias=bias_sb[:, 2 + ht:3 + ht])
                p2 = temps.tile([128, BG], F32, tag="p2", name=f"p2{g}_{ht}")
                nc.vector.tensor_mul(p2, et, twx[ht])
                ho = temps.tile([128, BG], F32, tag="ho", name=f"ho{g}_{ht}")
                nc.vector.tensor_add(ho, p1[ht], p2)
                nc.gpsimd.dma_start(out=out[g, :, ht, :], in_=ho)

        for g in range(N_BG):
            if g + 2 <= N_BG - 1:
                load_window(g + 2)
            for m in (st_map, xt_map, sts_map):
                for key in [k for k in m if k < g]:
                    del m[key]
            group(g)

    nc.compile()
    return nc


def _get_nc():
    key = (B, D_IN, H)
    if key not in _NC_CACHE:
        nc = bacc.Bacc("TRN2", target_bir_lowering=False, debug=False,
                       num_devices=NCORES)
        _NC_CACHE[key] = build(nc)
    return _NC_CACHE[key]


def _pack_acts(at):  # at: [D, B] transposed activations -> [n_bg, 128, KT, BG]
    d, b_ = at.shape
    return np.ascontiguousarray(
        at.reshape(KT, 128, N_BG, BG).transpose(2, 1, 0, 3)
    )


def _pack_w(wm):  # [D, h] -> [128, KT, h]
    d, h = wm.shape
    return np.ascontiguousarray(wm.reshape(KT, 128, h).transpose(1, 0, 2))


def make_in_maps(inputs):
    bf = ml_dtypes.bfloat16
    x = np.asarray(inputs["inputs"], dtype=np.float32)
    s = np.asarray(inputs["state"], dtype=np.float32)
    w = {k: np.asarray(inputs[k], dtype=np.float32)
         for k in ("theta_u_w", "theta_w_w", "eta_u_w", "eta_w_w", "wx_w")}
    bt_full = np.asarray(inputs["theta_w_b"], dtype=np.float32)
    be_full = np.asarray(inputs["eta_w_b"], dtype=np.float32)

    xt_p = _pack_acts(x.T.astype(bf))     # shared by all cores
    st_p = _pack_acts(s.T.astype(bf))

    in_maps = []
    for c in range(NCORES):
        hsl = slice(c * H_LOC, (c + 1) * H_LOC)
        # sts: this core's own hidden-state slice, [n_bg, 128, 2, BG]
        # element (g, p, ht, j) = state[g*BG+j, hsl.start + ht*128 + p]
        sts_c = np.ascontiguousarray(
            s[:, hsl].reshape(N_BG, BG, 2, 128).transpose(0, 3, 2, 1)
        ).astype(bf)
        bias_c = np.stack([
            bt_full[hsl][:128], bt_full[hsl][128:],
            be_full[hsl][:128], be_full[hsl][128:],
        ], axis=1).astype(np.float32)
        in_maps.append({
            "st": st_p,
            "xt": xt_p,
            "sts": sts_c,
            "wsu": _pack_w(np.concatenate(
                [w["theta_u_w"][:, hsl], w["eta_u_w"][:, hsl]], axis=1
            ).astype(bf)),
            "wsx": _pack_w(np.concatenate(
                [w["theta_w_w"][:, hsl], w["eta_w_w"][:, hsl]], axis=1
            ).astype(bf)),
            "wx": _pack_w(w["wx_w"][:, hsl].astype(bf)),
            "bias": np.ascontiguousarray(bias_c),
        })
    return in_maps


def kernel(**inputs):
    global LAST_RESULTS
    in_maps = make_in_maps(inputs)
    nc = _get_nc()
    res = run_bass_kernel_spmd(nc, in_maps, core_ids=list(range(NCORES)),
                               trace=TRACE)
    LAST_RESULTS = res

    h = np.empty((B, H), np.float32)
    for c in range(NCORES):
        o = res.results[c]["h_out"]  # [N_BG, 128, 2, BG]
        h[:, c * H_LOC:(c + 1) * H_LOC] = (
            o.transpose(0, 3, 2, 1).reshape(B, H_LOC)
        )
    return (h, h)


# revision 13
# speedup vs baseline: 1.1612x; 1.1612x over previous
"""CFN cell on 8 TRN2 NeuronCores — tensor-parallel over H, bf16 matmuls.

v4: weight-stationary orientation. Each core owns H_LOC=256 hidden
columns (2 tiles of 128). Stationary operand = weight chunk
[K=128, M=128 h-cols]; moving operand = transposed activations
[K=128, N=512 batch]. PSUM output is [h, batch], so

  * the theta/eta biases are per-partition -> folded into ACTIVATE,
  * tanh(state) comes from slices of the already-loaded st pack,
  * 1280 N=512 matmuls/core (vs 1536 mixed) — all at PE line rate.

All matmul operands are bf16 (host-converted): halves HBM traffic to
~39 MB/core and enables FWL weight loads; PSUM accumulation stays fp32
so the output error (~5e-4) is far inside the 2e-2 gate.
"""

import numpy as np
import ml_dtypes
from contextlib import ExitStack

import concourse.bass as bass
import concourse.mybir as mybir
import concourse.tile as tile
from concourse import bacc
from concourse.bass_utils import run_bass_kernel_spmd

F32 = mybir.dt.float32
BF16 = mybir.dt.bfloat16
AF = mybir.ActivationFunctionType

B, D_IN, H, NCORES = 4096, 2048, 2048, 8
H_LOC = H // NCORES          # 256 -> 2 output tiles of 128
BG = 512                     # batch-group width (PSUM bank limit)
N_BG = B // BG               # 8
KT = D_IN // 128             # 16 contraction chunks per operand side

TRACE = False
LAST_RESULTS = None
_NC_CACHE = {}


def build(nc):
    st = nc.dram_tensor("st", [N_BG, 128, KT, BG], BF16, kind="ExternalInput").ap()
    xt = nc.dram_tensor("xt", [N_BG, 128, KT, BG], BF16, kind="ExternalInput").ap()
    sts = nc.dram_tensor("sts", [N_BG, 128, 2, BG], BF16, kind="ExternalInput").ap()
    wsu = nc.dram_tensor("wsu", [128, KT, 512], BF16, kind="ExternalInput").ap()
    wsx = nc.dram_tensor("wsx", [128, KT, 512], BF16, kind="ExternalInput").ap()
    wx = nc.dram_tensor("wx", [128, KT, 256], BF16, kind="ExternalInput").ap()
    bias = nc.dram_tensor("bias", [128, 4], F32, kind="ExternalInput").ap()
    out = nc.dram_tensor("h_out", [N_BG, 128, 2, BG], BF16, kind="ExternalOutput").ap()

    with tile.TileContext(nc) as tc, ExitStack() as ctx:
        consts = ctx.enter_context(tc.tile_pool(name="consts", bufs=1))
        acts = ctx.enter_context(tc.tile_pool(name="acts", bufs=3))
        temps = ctx.enter_context(tc.tile_pool(name="temps", bufs=2))
        psum = ctx.enter_context(tc.tile_pool(name="psum", bufs=1, space="PSUM"))

        wsu_sb = consts.tile([128, KT, 512], BF16, tag="wsu")
        wsx_sb = consts.tile([128, KT, 512], BF16, tag="wsx")
        wx_sb = consts.tile([128, KT, 256], BF16, tag="wx")
        bias_sb = consts.tile([128, 4], F32, tag="bias")

        st_map, xt_map, sts_map = {}, {}, {}

        # DMA issue rings.  The 16 DMA engines pull from every active ring,
        # so aggregate HBM bandwidth scales with the number of rings kept
        # busy; round-robin in exact consumption order keeps delivery
        # aligned with what the PE needs next.  gpsimd is reserved for the
        # output writes in steady state (an out DMA waits on the epilogue
        # and would head-of-line-block window loads queued behind it).
        import itertools
        _rr = itertools.count()
        rings4 = [nc.sync, nc.gpsimd, nc.scalar]
        rings3 = [nc.sync, nc.scalar]

        def q4():
            return rings4[next(_rr) % 3]

        def load_window(g):
            xtw = acts.tile([128, KT, BG], BF16, tag="xtw", name=f"xtw{g}")
            for i, (c0, c1) in enumerate(((0, 8), (8, 16))):
                rings3[i].dma_start(out=xtw[:, c0:c1, :],
                                    in_=xt[g, :, c0:c1, :])
            xt_map[g] = xtw
            stw = acts.tile([128, KT, BG], BF16, tag="stw", name=f"stw{g}")
            for i, (c0, c1) in enumerate(((0, 8), (8, 16))):
                rings3[1 - i].dma_start(out=stw[:, c0:c1, :],
                                        in_=st[g, :, c0:c1, :])
            st_map[g] = stw
            stsw = acts.tile([128, 2, BG], BF16, tag="stsw", name=f"sts{g}")
            rings3[g % 2].dma_start(out=stsw, in_=sts[g])
            sts_map[g] = stsw

        # ── Startup choreography ────────────────────────────────────────
        # Consumption order: [xt0|wsx] pairs (input phase), wx, [st0|wsu]
        # pairs (state phase), then window 1.  Fine chunks at the front so
        # the first matmul can start ~1 chunk after DMA begins.
        fine = ((0, 1), (1, 4), (4, 8), (8, 12), (12, 16))
        xtw0 = acts.tile([128, KT, BG], BF16, tag="xtw", name="xtw0")
        xt_map[0] = xtw0
        for c0, c1 in fine:
            q4().dma_start(out=xtw0[:, c0:c1, :], in_=xt[0, :, c0:c1, :])
            q4().dma_start(out=wsx_sb[:, c0:c1, :], in_=wsx[:, c0:c1, :])
        q4().dma_start(out=wx_sb[:, 0:8, :], in_=wx[:, 0:8, :])
        q4().dma_start(out=wx_sb[:, 8:16, :], in_=wx[:, 8:16, :])
        stw0 = acts.tile([128, KT, BG], BF16, tag="stw", name="stw0")
        st_map[0] = stw0
        coarse = ((0, 4), (4, 8), (8, 12), (12, 16))
        for c0, c1 in coarse:
            q4().dma_start(out=stw0[:, c0:c1, :], in_=st[0, :, c0:c1, :])
            q4().dma_start(out=wsu_sb[:, c0:c1, :], in_=wsu[:, c0:c1, :])
        q4().dma_start(out=bias_sb, in_=bias)
        stsw0 = acts.tile([128, 2, BG], BF16, tag="stsw", name="sts0")
        q4().dma_start(out=stsw0, in_=sts[0])
        sts_map[0] = stsw0
        xtw1 = acts.tile([128, KT, BG], BF16, tag="xtw", name="xtw1")
        for c0, c1 in ((0, 8), (8, 16)):
            q4().dma_start(out=xtw1[:, c0:c1, :], in_=xt[1, :, c0:c1, :])
        xt_map[1] = xtw1
        stw1 = acts.tile([128, KT, BG], BF16, tag="stw", name="stw1")
        for c0, c1 in ((0, 8), (8, 16)):
            q4().dma_start(out=stw1[:, c0:c1, :], in_=st[1, :, c0:c1, :])
        st_map[1] = stw1
        stsw1 = acts.tile([128, 2, BG], BF16, tag="stsw", name="sts1")
        q4().dma_start(out=stsw1, in_=sts[1])
        sts_map[1] = stsw1

        def group(g):
            # Both h-tiles share every window / weight chunk, so running
            # them in one pass halves the DMA demand per PE-second — the
            # only thing that matters while the weights still stream in
            # (group 0 would otherwise need ~520 GB/s vs ~320 available).
            # ht-major sub-loops release each PSUM bank ~10 µs before the
            # group ends, so bufs=1 per tag never stalls a boundary.
            stw, xtw, stsw = st_map[g], xt_map[g], sts_map[g]
            hs = [slice(0, 128), slice(128, 256)]
            es = [slice(256, 384), slice(384, 512)]
            th_ps = [psum.tile([128, BG], F32, tag=f"th{ht}", bufs=1,
                               name=f"th{g}_{ht}") for ht in range(2)]
            et_ps = [psum.tile([128, BG], F32, tag=f"et{ht}", bufs=1,
                               name=f"et{g}_{ht}") for ht in range(2)]
            wx_ps = [psum.tile([128, BG], F32, tag=f"wx{ht}", bufs=1,
                               name=f"wx{g}_{ht}") for ht in range(2)]
            for ht in range(2):
                for k in range(KT):
                    nc.tensor.matmul(th_ps[ht], wsx_sb[:, k, hs[ht]],
                                     xtw[:, k, :], start=(k == 0), stop=False)
            for ht in range(2):
                for k in range(KT):
                    nc.tensor.matmul(et_ps[ht], wsx_sb[:, k, es[ht]],
                                     xtw[:, k, :], start=(k == 0), stop=False)
            for ht in range(2):
                for k in range(KT):
                    nc.tensor.matmul(wx_ps[ht], wx_sb[:, k, hs[ht]],
                                     xtw[:, k, :], start=(k == 0),
                                     stop=(k == KT - 1))
            for ht in range(2):
                for k in range(KT):
                    nc.tensor.matmul(th_ps[ht], wsu_sb[:, k, hs[ht]],
                                     stw[:, k, :], start=False,
                                     stop=(k == KT - 1))
            final = g == N_BG - 1
            for ht in range(2):
                if final and ht == 1:
                    break
                for k in range(KT):
                    nc.tensor.matmul(et_ps[ht], wsu_sb[:, k, es[ht]],
                                     stw[:, k, :], start=False,
                                     stop=(k == KT - 1))

            # epilogue, scalar-queue ops emitted in availability order so a
            # late PSUM (et) never head-of-line-blocks an earlier one
            ths, twx, th, p1 = [], [], [], []
            for ht in range(2):
                ths.append(temps.tile([128, BG], F32, tag="ths",
                                      name=f"ths{g}_{ht}"))
                nc.scalar.activation(ths[ht], stsw[:, ht, :], AF.Tanh)
            for ht in range(2):
                twx.append(temps.tile([128, BG], F32, tag="twx",
                                      name=f"twx{g}_{ht}"))
                nc.scalar.activation(twx[ht], wx_ps[ht], AF.Tanh)
            for ht in range(2):
                th.append(temps.tile([128, BG], F32, tag="th_s",
                                     name=f"ths_{g}_{ht}"))
                nc.scalar.activation(th[ht], th_ps[ht], AF.Sigmoid,
                                     bias=bias_sb[:, ht:ht + 1])
                p1.append(temps.tile([128, BG], F32, tag="p1",
                                     name=f"p1{g}_{ht}"))
                nc.vector.tensor_mul(p1[ht], th[ht], ths[ht])
            for ht in range(2):
                if final and ht == 1:
                    break
                et = temps.tile([128, BG], F32, tag="et_s", name=f"ets_{g}_{ht}")
                nc.scalar.activation(et, et_ps[ht], AF.Sigmoid,
                                     bias=bias_sb[:, 2 + ht:3 + ht])
                p2 = temps.tile([128, BG], F32, tag="p2", name=f"p2{g}_{ht}")
                nc.vector.tensor_mul(p2, et, twx[ht])
                ho = temps.tile([128, BG], BF16, tag="ho", name=f"ho{g}_{ht}")
                nc.vector.tensor_add(ho, p1[ht], p2)
                nc.gpsimd.dma_start(out=out[g, :, ht, :], in_=ho)

            if final:
                # last batch group: run the final ηs accumulation and its
                # epilogue in column halves so the ACT→mul→add→DMA chain
                # overlaps the remaining matmuls instead of trailing them
                for cs in (slice(0, BG // 2), slice(BG // 2, BG)):
                    for k in range(KT):
                        nc.tensor.matmul(et_ps[1][:, cs], wsu_sb[:, k, es[1]],
                                         stw[:, k, cs], start=False,
                                         stop=(k == KT - 1),
                                         skip_group_check=True)
                    et_h = temps.tile([128, BG // 2], F32, tag="et_h",
                                      name=f"eth{cs.start}")
                    nc.scalar.activation(et_h, et_ps[1][:, cs], AF.Sigmoid,
                                         bias=bias_sb[:, 3:4])
                    p2_h = temps.tile([128, BG // 2], F32, tag="p2_h",
                                      name=f"p2h{cs.start}")
                    nc.vector.tensor_mul(p2_h, et_h, twx[1][:, cs])
                    ho_h = temps.tile([128, BG // 2], BF16, tag="ho_h",
                                      name=f"hoh{cs.start}")
                    nc.vector.tensor_add(ho_h, p1[1][:, cs], p2_h)
                    nc.gpsimd.dma_start(out=out[g, :, 1, cs], in_=ho_h)

        for g in range(N_BG):
            if g + 2 <= N_BG - 1:
                load_window(g + 2)
            for m in (st_map, xt_map, sts_map):
                for key in [k for k in m if k < g]:
                    del m[key]
            group(g)

    nc.compile()
    return nc


def _get_nc():
    key = (B, D_IN, H)
    if key not in _NC_CACHE:
        nc = bacc.Bacc("TRN2", target_bir_lowering=False, debug=False,
                       num_devices=NCORES)
        _NC_CACHE[key] = build(nc)
    return _NC_CACHE[key]


def _pack_acts(at):  # at: [D, B] transposed activations -> [n_bg, 128, KT, BG]
    d, b_ = at.shape
    return np.ascontiguousarray(
        at.reshape(KT, 128, N_BG, BG).transpose(2, 1, 0, 3)
    )


def _pack_w(wm):  # [D, h] -> [128, KT, h]
    d, h = wm.shape
    return np.ascontiguousarray(wm.reshape(KT, 128, h).transpose(1, 0, 2))


def make_in_maps(inputs):
    bf = ml_dtypes.bfloat16
    x = np.asarray(inputs["inputs"], dtype=np.float32)
    s = np.asarray(inputs["state"], dtype=np.float32)
    w = {k: np.asarray(inputs[k], dtype=np.float32)
         for k in ("theta_u_w", "theta_w_w", "eta_u_w", "eta_w_w", "wx_w")}
    bt_full = np.asarray(inputs["theta_w_b"], dtype=np.float32)
    be_full = np.asarray(inputs["eta_w_b"], dtype=np.float32)

    xt_p = _pack_acts(x.T.astype(bf))     # shared by all cores
    st_p = _pack_acts(s.T.astype(bf))

    in_maps = []
    for c in range(NCORES):
        hsl = slice(c * H_LOC, (c + 1) * H_LOC)
        # sts: this core's own hidden-state slice, [n_bg, 128, 2, BG]
        # element (g, p, ht, j) = state[g*BG+j, hsl.start + ht*128 + p]
        sts_c = np.ascontiguousarray(
            s[:, hsl].reshape(N_BG, BG, 2, 128).transpose(0, 3, 2, 1)
        ).astype(bf)
        bias_c = np.stack([
            bt_full[hsl][:128], bt_full[hsl][128:],
            be_full[hsl][:128], be_full[hsl][128:],
        ], axis=1).astype(np.float32)
        in_maps.append({
            "st": st_p,
            "xt": xt_p,
            "sts": sts_c,
            "wsu": _pack_w(np.concatenate(
                [w["theta_u_w"][:, hsl], w["eta_u_w"][:, hsl]], axis=1
            ).astype(bf)),
            "wsx": _pack_w(np.concatenate(
                [w["theta_w_w"][:, hsl], w["eta_w_w"][:, hsl]], axis=1
            ).astype(bf)),
            "wx": _pack_w(w["wx_w"][:, hsl].astype(bf)),
            "bias": np.ascontiguousarray(bias_c),
        })
    return in_maps


def kernel(**inputs):
    global LAST_RESULTS
    in_maps = make_in_maps(inputs)
    nc = _get_nc()
    res = run_bass_kernel_spmd(nc, in_maps, core_ids=list(range(NCORES)),
                               trace=TRACE)
    LAST_RESULTS = res

    h = np.empty((B, H), np.float32)
    for c in range(NCORES):
        o = np.asarray(res.results[c]["h_out"], dtype=np.float32)
        h[:, c * H_LOC:(c + 1) * H_LOC] = (
            o.transpose(0, 3, 2, 1).reshape(B, H_LOC)
        )
    return (h, h)
